# revision 17
# baseline (speedup 1.0000x reference)
"""Two-layer GCN block on 8 Trainium2 NeuronCores (Bass/Tile).

out = GCNConv2(relu(GCNConv1(x, edge_index)))  with symmetric deg^-1/2 norm
and self-loops, matching PyG GCNConv defaults (b1 == 0 per the spec).

Strategy (memory-bound gather/segment-sum workload):
  - Nodes are sharded across 8 cores (12500 each, padded to 12544 = 98
    windows of 128 destinations). Edges live on the core owning their dst.
  - Linearity lets the layer-1 transform commute through aggregation:
        sum_s dinv_s (x W1)_s = (sum_s dinv_s x_s) W1
    so the device gathers rows of the pre-scaled table xs = dinv*x and
    applies W1 once per 128-dst window instead of once per node.
  - Edges are gathered with ONE dma_gather per (superchunk, src-block):
    buckets are (window, block) groups of 128 padded to the per-core max
    (dummy index 0 mid-stream, -1 run in each call's final bucket which the
    Q7 ucode trims at runtime), so the SWDGE fixed cost is paid 112 times
    instead of 784 and the schedule stays compile-time shared across cores.
  - The 0/1 selection matrix S (is_equal against an iota row, built from
    per-core rel-dst data with -1 marking padding slots) maps each group's
    128 gathered rows to their destinations; S for all groups of a
    (window, block) bucket is built in ONE all-bf16 DVE op (2x mode).
  - The "swapped" operand order yields the transposed aggregate [feat, dst]
    so the whole window epilogue ts2 = dinv^2 * (relu((aggT)^T W1) W2) runs
    with zero on-chip transposes (dst-side dinv folds through relu and W2
    because dinv > 0 and b1 = 0).
  - One AllGather exchanges the per-core ts2 shards (bf16).
  - Layer-2 aggregation gathers from the full ts2 with the SAME index /
    rel-dst tensors (loaded once, resident in SBUF) and writes
    out = dinv * agg + b2 for the core's own windows, one DMA per
    superchunk.
"""

import os
import sys

if "/opt/trn_rl_repo" not in sys.path:
    sys.path.insert(0, "/opt/trn_rl_repo")
os.environ.setdefault("NEURON_SCRATCHPAD_PAGE_SIZE", "512")

from dataclasses import dataclass

import ml_dtypes
import numpy as np

P = 128


@dataclass(frozen=True)
class Cfg:
    n: int  # number of real nodes
    n_cores: int = 8
    scw: int = 7  # windows per superchunk
    blk: int = 32768  # gather table rows per int16-addressable block

    @property
    def npc(self):
        return self.n // self.n_cores

    @property
    def wpc(self):
        return -(-self.npc // P)

    @property
    def ppc(self):
        return self.wpc * P

    @property
    def npad(self):
        return self.ppc * self.n_cores

    @property
    def nwin(self):
        return self.wpc * self.n_cores

    @property
    def nsc(self):
        assert self.wpc % self.scw == 0, (self.wpc, self.scw)
        return self.wpc // self.scw

    @property
    def nb(self):
        return -(-self.npad // self.blk)


CFG = Cfg(n=100000)

# pad each (superchunk, block) call's final bucket with -1 (ucode trims the
# trailing run at runtime — saves the padding's DMA transfer); False pads with
# dummy index 0 like the interior buckets
TRAILING_TRIM = False


# ----------------------------------------------------------------------------
# Host-side preprocessing: edge schedule shared by both layers.
# ----------------------------------------------------------------------------


def _preprocess(cfg: Cfg, edge_index: np.ndarray):
    n = cfg.n
    src = np.concatenate([edge_index[0], np.arange(n, dtype=np.int64)])
    dst = np.concatenate([edge_index[1], np.arange(n, dtype=np.int64)])
    deg = np.bincount(dst, minlength=n).astype(np.float32)

    def pad_id(v):
        c = v // cfg.npc
        return c * cfg.ppc + (v - c * cfg.npc)

    srcp = pad_id(src)
    dstp = pad_id(dst)
    core = dst // cfg.npc
    locw = (dstp % cfg.ppc) // P  # local window on owning core
    sgrp = locw // cfg.scw
    wj = locw % cfg.scw
    rel = (dstp % P).astype(np.float32)
    blk = srcp // cfg.blk
    lsrc = (srcp - blk * cfg.blk).astype(np.int16)

    # counts per (core, superchunk, block, window-in-superchunk)
    key = ((core * cfg.nsc + sgrp) * cfg.nb + blk) * cfg.scw + wj
    cnt = np.bincount(
        key, minlength=cfg.n_cores * cfg.nsc * cfg.nb * cfg.scw
    ).reshape(cfg.n_cores, cfg.nsc, cfg.nb, cfg.scw)
    G = (-(-cnt // P)).max(axis=0)  # [s, b, j] groups per bucket (shared)
    # bucket offsets within each (s, b) call's output stream (window-major)
    boff = np.zeros_like(G)
    boff[..., 1:] = np.cumsum(G, axis=-1)[..., :-1]
    boff *= P
    NI = G.sum(axis=-1) * P  # [s, b] idx slots per call (128-multiple)
    mcols = NI
    assert (G.sum(axis=(0, 1)) > 0).all(), "window with no groups"

    icoff = np.zeros(cfg.nsc * cfg.nb, dtype=np.int64)
    np.cumsum((NI // 16).ravel()[:-1], out=icoff[1:])
    icoff = icoff.reshape(cfg.nsc, cfg.nb)
    icols = int(NI.sum() // 16)

    # rd column layout in schedule order (s, j, b, g)
    rdoff = np.zeros((cfg.nsc, cfg.scw, cfg.nb), dtype=np.int64)
    acc = 0
    for s in range(cfg.nsc):
        for j_ in range(cfg.scw):
            for b in range(cfg.nb):
                rdoff[s, j_, b] = acc
                acc += int(G[s, b, j_])
    rcols = acc

    order = np.lexsort((lsrc, wj, blk, sgrp, core))
    key_sb = ((core * cfg.nsc + sgrp) * cfg.nb + blk) * cfg.scw + wj
    key_s = key_sb[order]
    rel_s = rel[order]
    lsrc_s = lsrc[order]

    bounds = np.searchsorted(
        key_s, np.arange(cfg.n_cores * cfg.nsc * cfg.nb * cfg.scw + 1)
    )
    # dummy index 0 for mid-stream padding; each call's final bucket pads
    # with -1 (trailing run, trimmed by the ucode at runtime)
    idx16 = np.zeros((cfg.n_cores, 16, icols), dtype=np.int16)
    lastj = np.zeros((cfg.nsc, cfg.nb), dtype=np.int64)
    for s in range(cfg.nsc):
        for b in range(cfg.nb):
            nz = np.nonzero(G[s, b])[0]
            lastj[s, b] = nz[-1] if nz.size else -1
    rd_all = np.full((cfg.n_cores, P, rcols), -1.0, dtype=np.float32)
    lanes = np.arange(P)[:, None]
    for c in range(cfg.n_cores):
        for s in range(cfg.nsc):
            for b in range(cfg.nb):
                for j_ in range(cfg.scw):
                    ng = int(G[s, b, j_])
                    if ng == 0:
                        continue
                    k = ((c * cfg.nsc + s) * cfg.nb + b) * cfg.scw + j_
                    loE, hiE = bounds[k], bounds[k + 1]
                    tc = hiE - loE
                    base = icoff[s, b] * 16 + boff[s, b, j_]
                    i = base + np.arange(tc)
                    idx16[c, i % 16, i // 16] = lsrc_s[loE:hiE]
                    if TRAILING_TRIM and j_ == lastj[s, b] and tc < ng * P:
                        ip = base + np.arange(tc, ng * P)
                        idx16[c, ip % 16, ip // 16] = -1
                    # rd columns for this bucket's groups
                    q = 128 * np.arange(ng)[None, :] + lanes  # [128, ng]
                    valid = q < tc
                    vals = np.where(
                        valid, rel_s[loE + np.clip(q, 0, max(tc - 1, 0))], -1.0
                    )
                    o = rdoff[s, j_, b]
                    rd_all[c, :, o : o + ng] = vals
    idx_all = np.tile(idx16, (1, 8, 1))

    degp = np.ones(cfg.npad, dtype=np.float32)
    degp[pad_id(np.arange(n))] = deg
    deg_all = degp.reshape(cfg.nwin, P).T.copy()  # [128, nwin] window-major

    return {
        "NI": NI,
        "mcols": mcols,
        "icoff": icoff,
        "G": G,
        "boff": boff,
        "rdoff": rdoff,
        "icols": icols,
        "rcols": rcols,
        "rd_all": rd_all,
        "idx_all": idx_all,
        "deg_all": deg_all,
        "degp": degp,
        "pad_id": pad_id,
    }


# ----------------------------------------------------------------------------
# Device program
# ----------------------------------------------------------------------------


def _build(
    cfg: Cfg,
    pre: dict,
    no_collective: bool = False,
    reps: int = 1,
):
    import concourse.bacc as bacc
    import concourse.bass as bass
    import concourse.mybir as mybir
    import concourse.tile as tile

    NI = pre["NI"]
    mcols = pre["mcols"]
    icoff = pre["icoff"]
    G = pre["G"]
    boff = pre["boff"]
    rdoff = pre["rdoff"]
    icols = pre["icols"]
    rcols = pre["rcols"]

    f32 = mybir.dt.float32
    bf16 = mybir.dt.bfloat16
    i16 = mybir.dt.int16
    EQ = mybir.AluOpType.is_equal
    MUL = mybir.AluOpType.mult
    ADD = mybir.AluOpType.add
    ACT_COPY = mybir.ActivationFunctionType.Copy
    ACT_RELU = mybir.ActivationFunctionType.Relu

    nc = bacc.Bacc("TRN2", target_bir_lowering=False, debug=False, num_swdge_queues=2)

    xs = nc.declare_dram_parameter("xs", [cfg.npad, P], bf16, isOutput=False)
    w1 = nc.declare_dram_parameter("w1", [P, P], bf16, isOutput=False)
    w2 = nc.declare_dram_parameter("w2", [P, P], bf16, isOutput=False)
    b2r = nc.declare_dram_parameter("b2r", [P, P], f32, isOutput=False)
    iota = nc.declare_dram_parameter("iota", [P, P], bf16, isOutput=False)
    deg_own = nc.declare_dram_parameter("deg_own", [P, cfg.wpc], f32, isOutput=False)
    idx_all = nc.declare_dram_parameter("idx_all", [P, icols], i16, isOutput=False)
    rd_all = nc.declare_dram_parameter("rd_all", [P, rcols], f32, isOutput=False)
    out = nc.declare_dram_parameter("out", [cfg.ppc, P], f32, isOutput=True)

    ts2s = nc.dram_tensor("ts2s", [cfg.ppc, P], bf16)
    ts2f = nc.dram_tensor("ts2f", [cfg.npad, P], bf16, addr_space="Shared")

    ts2s_r = ts2s.ap().rearrange("(t p) f -> p t f", p=P)
    out_r = out.ap().rearrange("(t p) f -> p t f", p=P)

    n_blk_rows = [min(cfg.blk, cfg.npad - b * cfg.blk) for b in range(cfg.nb)]
    mcols_max = [int(mcols[:, b].max()) for b in range(cfg.nb)]

    with tile.TileContext(nc) as tc:
        cpool = tc.tile_pool(name="const", bufs=1)
        cp = cpool.__enter__()
        w1_t = cp.tile([P, P], bf16)
        nc.sync.dma_start(w1_t[:], w1[:, :])
        w2_t = cp.tile([P, P], bf16)
        nc.sync.dma_start(w2_t[:], w2[:, :])
        b2_t = cp.tile([P, P], f32)
        nc.sync.dma_start(b2_t[:], b2r[:, :])
        iota_t = cp.tile([P, P], bf16)
        nc.sync.dma_start(iota_t[:], iota[:, :])

        # indices + rel-dst data are shared by both layers: load once, stay
        # resident for the whole program
        it_all = cp.tile([P, icols], i16)
        nc.sync.dma_start(it_all[:], idx_all[:, :])
        rdt_all = cp.tile([P, rcols], f32)
        nc.sync.dma_start(rdt_all[:], rd_all[:, :])

        dego_t = cp.tile([P, cfg.wpc], f32)
        nc.sync.dma_start(dego_t[:], deg_own[:, :])
        rcpo_t = cp.tile([P, cfg.wpc], f32)
        nc.vector.reciprocal(rcpo_t[:], dego_t[:])
        dinv_own = cp.tile([P, cfg.wpc], f32)
        nc.scalar.activation(dinv_own[:], rcpo_t[:], mybir.ActivationFunctionType.Sqrt)
        dinv2_own = cp.tile([P, cfg.wpc], f32)
        nc.vector.tensor_tensor(
            out=dinv2_own[:], in0=dinv_own[:], in1=dinv_own[:], op=MUL
        )

        def aggregate(table, layer1: bool, rep: int = 0):
            sfx = ("1" if layer1 else "2") + (f"r{rep}" if rep else "")
            with (
                tc.tile_pool(name=f"pg_msg{sfx}", bufs=2) as p_msg,
                tc.tile_pool(name=f"pg_s{sfx}", bufs=6) as p_s,
                tc.tile_pool(name=f"pg_eo{sfx}", bufs=3) as p_eo,
                tc.tile_pool(name=f"pg_osc{sfx}", bufs=2) as p_osc,
                tc.tile_pool(name=f"pg_ps{sfx}", bufs=2, space="PSUM") as p_ps,
                tc.tile_pool(name=f"pg_ps{sfx}b", bufs=2, space="PSUM") as p_ps2,
            ):
                for s in range(cfg.nsc):
                    mg = []
                    for b in range(cfg.nb):
                        ni_sb = int(NI[s, b])
                        mc_sb = int(mcols[s, b])
                        mt = p_msg.tile([P, mcols_max[b]], bf16, tag=f"mg{b}")
                        if ni_sb:
                            nc.gpsimd.dma_gather(
                                out_ap=mt[:, :mc_sb].rearrange(
                                    "p (g e) -> p g e", e=P
                                ),
                                in_ap=table[
                                    b * cfg.blk : b * cfg.blk + n_blk_rows[b], :
                                ],
                                idxs_ap=it_all[
                                    :, icoff[s, b] : icoff[s, b] + ni_sb // 16
                                ],
                                num_idxs=ni_sb,
                                num_idxs_reg=ni_sb,
                                elem_size=P,
                                single_packet=False,
                                queue_num=(s * cfg.nb + b) % 2,
                            )
                        mg.append(mt)

                    osc = p_osc.tile(
                        [P, cfg.scw * P], bf16 if layer1 else f32, tag="osc"
                    )
                    for j_ in range(cfg.scw):
                        j = s * cfg.scw + j_
                        gw = int(G[s, :, j_].sum())
                        ps = p_ps.tile([P, P], f32, space="PSUM", tag="agg")
                        k = 0
                        for b in range(cfg.nb):
                            ng = int(G[s, b, j_])
                            for lg in range(ng):
                                o = int(rdoff[s, j_, b]) + lg
                                st = p_s.tile([P, P], bf16, tag="sel")
                                nc.vector.tensor_scalar(
                                    out=st[:],
                                    in0=iota_t[:],
                                    scalar1=rdt_all[:, o : o + 1],
                                    scalar2=None,
                                    op0=EQ,
                                )
                                c0 = int(boff[s, b, j_]) + lg * P
                                if layer1:
                                    nc.tensor.matmul(
                                        ps[:],
                                        lhsT=mg[b][:, c0 : c0 + P],
                                        rhs=st[:, :],
                                        start=(k == 0),
                                        stop=(k == gw - 1),
                                    )
                                else:
                                    nc.tensor.matmul(
                                        ps[:],
                                        lhsT=st[:, :],
                                        rhs=mg[b][:, c0 : c0 + P],
                                        start=(k == 0),
                                        stop=(k == gw - 1),
                                    )
                                k += 1
                        oj = slice(j_ * P, (j_ + 1) * P)
                        if layer1:
                            # ps = agg1^T [feat_in, dst]
                            c1 = p_eo.tile([P, P], bf16, tag="c1")
                            nc.scalar.activation(c1[:], ps[:], ACT_COPY)
                            ps2 = p_ps2.tile([P, P], f32, space="PSUM", tag="t1")
                            nc.tensor.matmul(
                                ps2[:], lhsT=w1_t[:], rhs=c1[:], start=True, stop=True
                            )
                            # ps2 = (agg1 W1)^T [feat_h, dst]
                            rt = p_eo.tile([P, P], bf16, tag="rt")
                            nc.scalar.activation(rt[:], ps2[:], ACT_RELU)
                            ps3 = p_ps.tile([P, P], f32, space="PSUM", tag="mm2")
                            nc.tensor.matmul(
                                ps3[:], lhsT=rt[:], rhs=w2_t[:], start=True, stop=True
                            )
                            # ts2 = dinv^2 * (relu(agg1 W1) W2)  [dst, feat2]
                            nc.scalar.activation(
                                osc[:, oj], ps3[:], ACT_COPY,
                                scale=dinv2_own[:, j : j + 1],
                            )
                        else:
                            o1 = p_eo.tile([P, P], f32, tag="o1")
                            nc.scalar.activation(
                                o1[:], ps[:], ACT_COPY, scale=dinv_own[:, j : j + 1]
                            )
                            nc.vector.tensor_tensor(
                                out=osc[:, oj], in0=o1[:], in1=b2_t[:], op=ADD
                            )
                    dst_r = ts2s_r if layer1 else out_r
                    nc.sync.dma_start(
                        dst_r[:, s * cfg.scw : (s + 1) * cfg.scw, :],
                        osc[:].rearrange("p (t f) -> p t f", f=P),
                    )

        for rep in range(reps):
            # ---- phase B: layer-1 aggregation (gathers from xs) + ts2 ------
            aggregate(xs[:, :], layer1=True, rep=rep)

            # ---- exchange --------------------------------------------------
            if not no_collective:
                nc.gpsimd.collective_compute(
                    "AllGather",
                    mybir.AluOpType.bypass,
                    replica_groups=[list(range(cfg.n_cores))],
                    ins=[ts2s[:, :]],
                    outs=[ts2f[:, :]],
                )

            # ---- phase C: layer-2 aggregation + output ---------------------
            aggregate(xs[:, :] if no_collective else ts2f.ap(), layer1=False, rep=rep)

        cpool.__exit__(None, None, None)

    nc.compile()
    return nc


# ----------------------------------------------------------------------------
# Entry point
# ----------------------------------------------------------------------------

_CACHE = {}


def _prep_inputs(cfg: Cfg, pre, x, W1, W2, b2):
    n = cfg.n
    dinv = 1.0 / np.sqrt(pre["degp"])  # padded slots have deg=1
    xsp = np.zeros((cfg.npad, P), dtype=np.float32)
    xsp[pre["pad_id"](np.arange(n))] = np.asarray(x, np.float32)
    xsp *= dinv[:, None]
    xs = xsp.astype(ml_dtypes.bfloat16)

    iota = np.broadcast_to(np.arange(P, dtype=np.float32), (P, P)).astype(
        ml_dtypes.bfloat16
    )
    in_maps = []
    for c in range(cfg.n_cores):
        in_maps.append(
            {
                "xs": xs,
                "w1": np.asarray(W1, np.float32).astype(ml_dtypes.bfloat16),
                "w2": np.asarray(W2, np.float32).astype(ml_dtypes.bfloat16),
                "b2r": np.broadcast_to(np.asarray(b2, np.float32), (P, P)).copy(),
                "iota": np.ascontiguousarray(iota),
                "deg_own": pre["deg_all"][:, c * cfg.wpc : (c + 1) * cfg.wpc],
                "idx_all": pre["idx_all"][c],
                "rd_all": np.ascontiguousarray(pre["rd_all"][c]),
            }
        )
    return in_maps


def _get_nc(cfg: Cfg, pre):
    key = (
        cfg,
        pre["icols"],
        pre["rcols"],
        pre["NI"].tobytes(),
        pre["G"].tobytes(),
    )
    if key not in _CACHE:
        _CACHE[key] = _build(cfg, pre)
    return _CACHE[key]


def _kernel_impl(cfg: Cfg, x, edge_index, W1, b1, W2, b2):
    from concourse.bass_utils import run_bass_kernel_spmd

    assert np.allclose(b1, 0.0), "kernel assumes b1 == 0 (spec fill: zeros)"

    pre = _preprocess(cfg, np.asarray(edge_index, dtype=np.int64))
    nc = _get_nc(cfg, pre)
    in_maps = _prep_inputs(cfg, pre, x, W1, W2, b2)

    res = run_bass_kernel_spmd(nc, in_maps, list(range(cfg.n_cores)))
    parts = [res.results[c]["out"][: cfg.npc] for c in range(cfg.n_cores)]
    return np.concatenate(parts, axis=0)


def kernel(x, edge_index, W1, b1, W2, b2):
    return _kernel_impl(CFG, x, edge_index, W1, b1, W2, b2)


# ----------------------------------------------------------------------------
# Steady-state timing support (no NTFF profiling under this axon client: we
# time repeated executions with device-resident inputs and subtract the
# dispatch floor measured with a null kernel).
# ----------------------------------------------------------------------------


def _make_runner(nc, n_cores):
    import jax
    from jax.sharding import Mesh, NamedSharding, PartitionSpec
    from jax.experimental.shard_map import shard_map

    from concourse import bass2jax, mybir

    bass2jax.install_neuronx_cc_hook()
    partition_name = nc.partition_id_tensor.name if nc.partition_id_tensor else None
    in_names, out_names, out_avals, zero_outs = [], [], [], []
    for alloc in nc.m.functions[0].allocations:
        if not isinstance(alloc, mybir.MemoryLocationSet):
            continue
        name = alloc.memorylocations[0].name
        if alloc.kind == "ExternalInput":
            if name != partition_name:
                in_names.append(name)
        elif alloc.kind == "ExternalOutput":
            shape = tuple(alloc.tensor_shape)
            dtype = mybir.dt.np(alloc.dtype)
            out_names.append(name)
            out_avals.append(jax.core.ShapedArray(shape, dtype))
            zero_outs.append(np.zeros(shape, dtype))
    n_params = len(in_names)
    all_in_names = list(in_names) + list(out_names)
    if partition_name is not None:
        all_in_names.append(partition_name)

    def _body(*args):
        operands = list(args)
        if partition_name is not None:
            operands.append(bass2jax.partition_id_tensor())
        outs = bass2jax._bass_exec_p.bind(
            *operands,
            out_avals=tuple(out_avals),
            in_names=tuple(all_in_names),
            out_names=tuple(out_names),
            lowering_input_output_aliases=(),
            sim_require_finite=True,
            sim_require_nnan=True,
            nc=nc,
        )
        return tuple(outs)

    devices = jax.devices()[:n_cores]
    mesh = Mesh(np.asarray(devices), ("core",))
    in_specs = (PartitionSpec("core"),) * (n_params + len(out_names))
    out_specs = (PartitionSpec("core"),) * len(out_names)
    fn = jax.jit(
        shard_map(
            _body, mesh=mesh, in_specs=in_specs, out_specs=out_specs, check_rep=False
        ),
        keep_unused=True,
    )
    sharding = NamedSharding(mesh, PartitionSpec("core"))

    def run(in_maps, iters=1):
        import time as _t

        concat = [
            np.concatenate([np.asarray(in_maps[c][n]) for c in range(n_cores)], axis=0)
            for n in in_names
        ]
        concat += [
            np.zeros((n_cores * z.shape[0], *z.shape[1:]), z.dtype) for z in zero_outs
        ]
        dev_in = [jax.device_put(a, sharding) for a in concat]
        outs = fn(*dev_in)
        jax.block_until_ready(outs)
        times = []
        for _ in range(iters):
            t0 = _t.perf_counter()
            outs = fn(*dev_in)
            jax.block_until_ready(outs)
            times.append(_t.perf_counter() - t0)
        return outs, out_names, out_avals, times

    return run


def time_kernel(x, edge_index, W1, b1, W2, b2, iters=30, reps=9):
    cfg = CFG
    pre = _preprocess(cfg, np.asarray(edge_index, dtype=np.int64))
    in_maps = _prep_inputs(cfg, pre, x, W1, W2, b2)

    nc1 = _get_nc(cfg, pre)
    run1 = _make_runner(nc1, cfg.n_cores)
    _, _, _, t1 = run1(in_maps, iters=iters)

    ncR = _build(cfg, pre, reps=reps)
    runR = _make_runner(ncR, cfg.n_cores)
    _, _, _, tR = runR(in_maps, iters=iters)

    est = (min(tR) - min(t1)) / (reps - 1)
    print(
        f"(x1: min {min(t1)*1e3:.3f} med {sorted(t1)[len(t1)//2]*1e3:.3f} ms; "
        f"x{reps}: min {min(tR)*1e3:.3f} med {sorted(tR)[len(tR)//2]*1e3:.3f} ms)"
    )
    return est * 1e9


# revision 28
# speedup vs baseline: 1.5917x; 1.5917x over previous
"""Two-layer GCN block on 8 Trainium2 NeuronCores (Bass/Tile).

out = GCNConv2(relu(GCNConv1(x, edge_index)))  with symmetric deg^-1/2 norm
and self-loops, matching PyG GCNConv defaults (b1 == 0 per the spec).

Strategy (memory-bound gather/segment-sum workload):
  - Nodes are sharded across 8 cores (12500 each, padded to 12544 = 98
    windows of 128 destinations). Edges live on the core owning their dst.
  - Linearity lets the layer-1 transform commute through aggregation:
        sum_s dinv_s (x W1)_s = (sum_s dinv_s x_s) W1
    so the device gathers rows of the pre-scaled table xs = dinv*x and
    applies W1 once per 128-dst window instead of once per node.
  - Edges are gathered with ONE dma_gather per (superchunk, src-block):
    buckets are (window, block) groups of 128 padded to the per-core max
    (dummy index 0 mid-stream, -1 run in each call's final bucket which the
    Q7 ucode trims at runtime), so the SWDGE fixed cost is paid 112 times
    instead of 784 and the schedule stays compile-time shared across cores.
  - The 0/1 selection matrix S (is_equal against an iota row, built from
    per-core rel-dst data with -1 marking padding slots) maps each group's
    128 gathered rows to their destinations; S for all groups of a
    (window, block) bucket is built in ONE all-bf16 DVE op (2x mode).
  - The "swapped" operand order yields the transposed aggregate [feat, dst]
    so the whole window epilogue ts2 = dinv^2 * (relu((aggT)^T W1) W2) runs
    with zero on-chip transposes (dst-side dinv folds through relu and W2
    because dinv > 0 and b1 = 0).
  - One AllGather exchanges the per-core ts2 shards (bf16).
  - Layer-2 aggregation gathers from the full ts2 with the SAME index /
    rel-dst tensors (loaded once, resident in SBUF) and writes
    out = dinv * agg + b2 for the core's own windows, one DMA per
    superchunk.
"""

import os
import sys

if "/opt/trn_rl_repo" not in sys.path:
    sys.path.insert(0, "/opt/trn_rl_repo")
os.environ.setdefault("NEURON_SCRATCHPAD_PAGE_SIZE", "512")

from dataclasses import dataclass

import ml_dtypes
import numpy as np

P = 128


@dataclass(frozen=True)
class Cfg:
    n: int  # number of real nodes
    n_cores: int = 8
    scw: int = 7  # windows per superchunk
    blk: int = 32768  # gather table rows per int16-addressable block

    @property
    def npc(self):
        return self.n // self.n_cores

    @property
    def wpc(self):
        return -(-self.npc // P)

    @property
    def ppc(self):
        return self.wpc * P

    @property
    def npad(self):
        return self.ppc * self.n_cores

    @property
    def nwin(self):
        return self.wpc * self.n_cores

    @property
    def nsc(self):
        assert self.wpc % self.scw == 0, (self.wpc, self.scw)
        return self.wpc // self.scw

    @property
    def nb(self):
        return -(-self.npad // self.blk)


CFG = Cfg(n=100000)

# Pad each (window, block) bucket with -1 and pass the per-core real count
# via num_idxs_reg: the decode layer reserves ring space from the register and
# the Q7 ucode trims the trailing -1 run from the index data, so padding costs
# no descriptors / no DMA. The register MUST equal the post-trim count or the
# ring bookkeeping desyncs (garbage descriptors -> device hang).
# Message-tile buffers are memset once per pass before the superchunk loop:
# trimmed calls leave unwritten slots, and uninitialized SBUF can hold NaN bit
# patterns that survive the S-matrix masking (NaN * 0 = NaN in the matmul).
TRAILING_TRIM = True
WARMUP_SC = 0
SINGLE_PACKET = False


# ----------------------------------------------------------------------------
# Host-side preprocessing: edge schedule shared by both layers.
# ----------------------------------------------------------------------------


def _preprocess(cfg: Cfg, edge_index: np.ndarray):
    n = cfg.n
    src = np.concatenate([edge_index[0], np.arange(n, dtype=np.int64)])
    dst = np.concatenate([edge_index[1], np.arange(n, dtype=np.int64)])
    deg = np.bincount(dst, minlength=n).astype(np.float32)

    def pad_id(v):
        c = v // cfg.npc
        return c * cfg.ppc + (v - c * cfg.npc)

    srcp = pad_id(src)
    dstp = pad_id(dst)
    core = dst // cfg.npc
    locw = (dstp % cfg.ppc) // P  # local window on owning core
    sgrp = locw // cfg.scw
    wj = locw % cfg.scw
    rel = (dstp % P).astype(np.float32)
    blk = srcp // cfg.blk
    lsrc = (srcp - blk * cfg.blk).astype(np.int16)

    # counts per (core, superchunk, block, window-in-superchunk)
    key = ((core * cfg.nsc + sgrp) * cfg.nb + blk) * cfg.scw + wj
    cnt = np.bincount(
        key, minlength=cfg.n_cores * cfg.nsc * cfg.nb * cfg.scw
    ).reshape(cfg.n_cores, cfg.nsc, cfg.nb, cfg.scw)
    G = (-(-cnt // P)).max(axis=0)  # [s, b, j] groups per bucket (shared)
    # bucket offsets within each (s, b) call's output stream (window-major)
    boff = np.zeros_like(G)
    boff[..., 1:] = np.cumsum(G, axis=-1)[..., :-1]
    boff *= P
    NI = G.sum(axis=-1) * P  # [s, b] idx slots per call (128-multiple)
    mcols = NI
    assert (G.sum(axis=(0, 1)) > 0).all(), "window with no groups"

    icoff = np.zeros(cfg.nsc * cfg.nb, dtype=np.int64)
    np.cumsum((NI // 16).ravel()[:-1], out=icoff[1:])
    icoff = icoff.reshape(cfg.nsc, cfg.nb)
    icols = int(NI.sum() // 16)

    # rd column layout in schedule order (s, j, b, g)
    rdoff = np.zeros((cfg.nsc, cfg.scw, cfg.nb), dtype=np.int64)
    acc = 0
    for s in range(cfg.nsc):
        for j_ in range(cfg.scw):
            for b in range(cfg.nb):
                rdoff[s, j_, b] = acc
                acc += int(G[s, b, j_])
    rcols = acc

    order = np.lexsort((lsrc, wj, blk, sgrp, core))
    key_sb = ((core * cfg.nsc + sgrp) * cfg.nb + blk) * cfg.scw + wj
    key_s = key_sb[order]
    rel_s = rel[order]
    lsrc_s = lsrc[order]

    bounds = np.searchsorted(
        key_s, np.arange(cfg.n_cores * cfg.nsc * cfg.nb * cfg.scw + 1)
    )
    # per-bucket padding: -1 (trimmed via num_idxs_reg) after warmup, dummy
    # index 0 during the warmup superchunks (full-length gathers)
    idx16 = np.zeros((cfg.n_cores, 16, icols), dtype=np.int16)
    ncalls = cfg.nsc * cfg.nb * cfg.scw
    gcnt = np.zeros((cfg.n_cores, ncalls), dtype=np.int32)
    rd_all = np.full((cfg.n_cores, P, rcols), -1.0, dtype=np.float32)
    lanes = np.arange(P)[:, None]
    for c in range(cfg.n_cores):
        for s in range(cfg.nsc):
            trim = TRAILING_TRIM and s >= WARMUP_SC
            for b in range(cfg.nb):
                for j_ in range(cfg.scw):
                    ng = int(G[s, b, j_])
                    if ng == 0:
                        continue
                    k = ((c * cfg.nsc + s) * cfg.nb + b) * cfg.scw + j_
                    loE, hiE = bounds[k], bounds[k + 1]
                    tc = hiE - loE
                    call = (s * cfg.nb + b) * cfg.scw + j_
                    gcnt[c, call] = tc if trim else ng * P
                    base = icoff[s, b] * 16 + boff[s, b, j_]
                    i = base + np.arange(tc)
                    idx16[c, i % 16, i // 16] = lsrc_s[loE:hiE]
                    if trim and tc < ng * P:
                        ip = base + np.arange(tc, ng * P)
                        idx16[c, ip % 16, ip // 16] = -1
                    # rd columns for this bucket's groups
                    q = 128 * np.arange(ng)[None, :] + lanes  # [128, ng]
                    valid = q < tc
                    vals = np.where(
                        valid, rel_s[loE + np.clip(q, 0, max(tc - 1, 0))], -1.0
                    )
                    o = rdoff[s, j_, b]
                    rd_all[c, :, o : o + ng] = vals
    idx_all = np.tile(idx16, (1, 8, 1))

    degp = np.ones(cfg.npad, dtype=np.float32)
    degp[pad_id(np.arange(n))] = deg
    deg_all = degp.reshape(cfg.nwin, P).T.copy()  # [128, nwin] window-major

    return {
        "NI": NI,
        "mcols": mcols,
        "icoff": icoff,
        "G": G,
        "boff": boff,
        "gcnt": gcnt,
        "rdoff": rdoff,
        "icols": icols,
        "rcols": rcols,
        "rd_all": rd_all,
        "idx_all": idx_all,
        "deg_all": deg_all,
        "degp": degp,
        "pad_id": pad_id,
    }


# ----------------------------------------------------------------------------
# Device program
# ----------------------------------------------------------------------------


def _build(
    cfg: Cfg,
    pre: dict,
    no_collective: bool = False,
    reps: int = 1,
):
    import concourse.bacc as bacc
    import concourse.bass as bass
    import concourse.mybir as mybir
    import concourse.tile as tile

    NI = pre["NI"]
    mcols = pre["mcols"]
    icoff = pre["icoff"]
    G = pre["G"]
    boff = pre["boff"]
    rdoff = pre["rdoff"]
    icols = pre["icols"]
    rcols = pre["rcols"]

    f32 = mybir.dt.float32
    bf16 = mybir.dt.bfloat16
    i16 = mybir.dt.int16
    EQ = mybir.AluOpType.is_equal
    MUL = mybir.AluOpType.mult
    ADD = mybir.AluOpType.add
    ACT_COPY = mybir.ActivationFunctionType.Copy
    ACT_RELU = mybir.ActivationFunctionType.Relu

    nc = bacc.Bacc("TRN2", target_bir_lowering=False, debug=False, num_swdge_queues=2)

    i32 = mybir.dt.int32
    ncalls = cfg.nsc * cfg.nb * cfg.scw

    xs = nc.declare_dram_parameter("xs", [cfg.npad, P], bf16, isOutput=False)
    gcnt = nc.declare_dram_parameter("gcnt", [1, ncalls], i32, isOutput=False)
    w1 = nc.declare_dram_parameter("w1", [P, P], bf16, isOutput=False)
    w2 = nc.declare_dram_parameter("w2", [P, P], bf16, isOutput=False)
    b2r = nc.declare_dram_parameter("b2r", [P, P], f32, isOutput=False)
    iota = nc.declare_dram_parameter("iota", [P, P], bf16, isOutput=False)
    deg_own = nc.declare_dram_parameter("deg_own", [P, cfg.wpc], f32, isOutput=False)
    idx_all = nc.declare_dram_parameter("idx_all", [P, icols], i16, isOutput=False)
    rd_all = nc.declare_dram_parameter("rd_all", [P, rcols], f32, isOutput=False)
    out = nc.declare_dram_parameter("out", [cfg.ppc, P], f32, isOutput=True)

    ts2s = nc.dram_tensor("ts2s", [cfg.ppc, P], bf16)
    ts2f = nc.dram_tensor("ts2f", [cfg.npad, P], bf16, addr_space="Shared")

    ts2s_r = ts2s.ap().rearrange("(t p) f -> p t f", p=P)
    out_r = out.ap().rearrange("(t p) f -> p t f", p=P)

    n_blk_rows = [min(cfg.blk, cfg.npad - b * cfg.blk) for b in range(cfg.nb)]
    mcols_max = [int(mcols[:, b].max()) for b in range(cfg.nb)]

    with tile.TileContext(nc) as tc:
        cpool = tc.tile_pool(name="const", bufs=1)
        cp = cpool.__enter__()
        w1_t = cp.tile([P, P], bf16)
        nc.sync.dma_start(w1_t[:], w1[:, :])
        w2_t = cp.tile([P, P], bf16)
        nc.sync.dma_start(w2_t[:], w2[:, :])
        b2_t = cp.tile([P, P], f32)
        nc.sync.dma_start(b2_t[:], b2r[:, :])
        iota_t = cp.tile([P, P], bf16)
        nc.sync.dma_start(iota_t[:], iota[:, :])

        # indices + rel-dst data are shared by both layers: load once, stay
        # resident for the whole program
        it_all = cp.tile([P, icols], i16)
        nc.sync.dma_start(it_all[:], idx_all[:, :])
        rdt_all = cp.tile([P, rcols], f32)
        nc.sync.dma_start(rdt_all[:], rd_all[:, :])
        gcnt_t = cp.tile([1, ncalls], i32)
        nc.sync.dma_start(gcnt_t[:], gcnt[:, :])

        dego_t = cp.tile([P, cfg.wpc], f32)
        nc.sync.dma_start(dego_t[:], deg_own[:, :])
        rcpo_t = cp.tile([P, cfg.wpc], f32)
        nc.vector.reciprocal(rcpo_t[:], dego_t[:])
        dinv_own = cp.tile([P, cfg.wpc], f32)
        nc.scalar.activation(dinv_own[:], rcpo_t[:], mybir.ActivationFunctionType.Sqrt)
        dinv2_own = cp.tile([P, cfg.wpc], f32)
        nc.vector.tensor_tensor(
            out=dinv2_own[:], in0=dinv_own[:], in1=dinv_own[:], op=MUL
        )

        def aggregate(table, layer1: bool, rep: int = 0):
            sfx = ("1" if layer1 else "2") + (f"r{rep}" if rep else "")
            with (
                tc.tile_pool(name=f"pg_msg{sfx}", bufs=2) as p_msg,
                tc.tile_pool(name=f"pg_s{sfx}", bufs=6) as p_s,
                tc.tile_pool(name=f"pg_eo{sfx}", bufs=3) as p_eo,
                tc.tile_pool(name=f"pg_osc{sfx}", bufs=2) as p_osc,
                tc.tile_pool(name=f"pg_ps{sfx}", bufs=2, space="PSUM") as p_ps,
                tc.tile_pool(name=f"pg_ps{sfx}b", bufs=2, space="PSUM") as p_ps2,
            ):
                # define every byte of both rotation buffers of each message
                # tile before any trimmed gather can leave stale SBUF visible
                for _w in range(2):
                    for b in range(cfg.nb):
                        mt = p_msg.tile([P, mcols_max[b]], bf16, tag=f"mg{b}")
                        nc.vector.memset(mt[:], 0.0)
                for s in range(cfg.nsc):
                    mg = []
                    for b in range(cfg.nb):
                        mt = p_msg.tile([P, mcols_max[b]], bf16, tag=f"mg{b}")
                        mg.append(mt)
                        if not int(NI[s, b]):
                            continue
                        # per-core real bucket counts -> num_idxs_reg (one
                        # multi-register load per (s, b) on the Pool engine)
                        cbase = (s * cfg.nb + b) * cfg.scw
                        _, vals = nc.values_load_multi_w_load_instructions(
                            gcnt_t[0:1, cbase : cbase + cfg.scw],
                            engines=[mybir.EngineType.Pool],
                            skip_runtime_bounds_check=True,
                        )
                        for j_ in range(cfg.scw):
                            ng = int(G[s, b, j_])
                            if ng == 0:
                                continue
                            c0 = int(boff[s, b, j_])
                            i0 = icoff[s, b] + c0 // 16
                            nc.gpsimd.dma_gather(
                                out_ap=mt[:, c0 : c0 + ng * P].rearrange(
                                    "p (g e) -> p g e", e=P
                                ),
                                in_ap=table[
                                    b * cfg.blk : b * cfg.blk + n_blk_rows[b], :
                                ],
                                idxs_ap=it_all[:, i0 : i0 + ng * 8],
                                num_idxs=ng * P,
                                num_idxs_reg=vals[j_],
                                elem_size=P,
                                single_packet=SINGLE_PACKET,
                                queue_num=(s * cfg.nb + b) % 2,
                            )

                    osc = p_osc.tile(
                        [P, cfg.scw * P], bf16 if layer1 else f32, tag="osc"
                    )
                    for j_ in range(cfg.scw):
                        j = s * cfg.scw + j_
                        gw = int(G[s, :, j_].sum())
                        ps = p_ps.tile([P, P], f32, space="PSUM", tag="agg")
                        k = 0
                        for b in range(cfg.nb):
                            ng = int(G[s, b, j_])
                            for lg in range(ng):
                                o = int(rdoff[s, j_, b]) + lg
                                st = p_s.tile([P, P], bf16, tag="sel")
                                nc.vector.tensor_scalar(
                                    out=st[:],
                                    in0=iota_t[:],
                                    scalar1=rdt_all[:, o : o + 1],
                                    scalar2=None,
                                    op0=EQ,
                                )
                                c0 = int(boff[s, b, j_]) + lg * P
                                if layer1:
                                    nc.tensor.matmul(
                                        ps[:],
                                        lhsT=mg[b][:, c0 : c0 + P],
                                        rhs=st[:, :],
                                        start=(k == 0),
                                        stop=(k == gw - 1),
                                    )
                                else:
                                    nc.tensor.matmul(
                                        ps[:],
                                        lhsT=st[:, :],
                                        rhs=mg[b][:, c0 : c0 + P],
                                        start=(k == 0),
                                        stop=(k == gw - 1),
                                    )
                                k += 1
                        oj = slice(j_ * P, (j_ + 1) * P)
                        if layer1:
                            # ps = agg1^T [feat_in, dst]
                            c1 = p_eo.tile([P, P], bf16, tag="c1")
                            nc.scalar.activation(c1[:], ps[:], ACT_COPY)
                            ps2 = p_ps2.tile([P, P], f32, space="PSUM", tag="t1")
                            nc.tensor.matmul(
                                ps2[:], lhsT=w1_t[:], rhs=c1[:], start=True, stop=True
                            )
                            # ps2 = (agg1 W1)^T [feat_h, dst]
                            rt = p_eo.tile([P, P], bf16, tag="rt")
                            nc.scalar.activation(rt[:], ps2[:], ACT_RELU)
                            ps3 = p_ps.tile([P, P], f32, space="PSUM", tag="mm2")
                            nc.tensor.matmul(
                                ps3[:], lhsT=rt[:], rhs=w2_t[:], start=True, stop=True
                            )
                            # ts2 = dinv^2 * (relu(agg1 W1) W2)  [dst, feat2]
                            nc.scalar.activation(
                                osc[:, oj], ps3[:], ACT_COPY,
                                scale=dinv2_own[:, j : j + 1],
                            )
                        else:
                            o1 = p_eo.tile([P, P], f32, tag="o1")
                            nc.scalar.activation(
                                o1[:], ps[:], ACT_COPY, scale=dinv_own[:, j : j + 1]
                            )
                            nc.vector.tensor_tensor(
                                out=osc[:, oj], in0=o1[:], in1=b2_t[:], op=ADD
                            )
                    dst_r = ts2s_r if layer1 else out_r
                    nc.sync.dma_start(
                        dst_r[:, s * cfg.scw : (s + 1) * cfg.scw, :],
                        osc[:].rearrange("p (t f) -> p t f", f=P),
                    )

        for rep in range(reps):
            # ---- phase B: layer-1 aggregation (gathers from xs) + ts2 ------
            aggregate(xs[:, :], layer1=True, rep=rep)

            # ---- exchange --------------------------------------------------
            if not no_collective:
                nc.gpsimd.collective_compute(
                    "AllGather",
                    mybir.AluOpType.bypass,
                    replica_groups=[list(range(cfg.n_cores))],
                    ins=[ts2s[:, :]],
                    outs=[ts2f[:, :]],
                )

            # ---- phase C: layer-2 aggregation + output ---------------------
            aggregate(xs[:, :] if no_collective else ts2f.ap(), layer1=False, rep=rep)

        cpool.__exit__(None, None, None)

    nc.compile()
    return nc


# ----------------------------------------------------------------------------
# Entry point
# ----------------------------------------------------------------------------

_CACHE = {}


def _prep_inputs(cfg: Cfg, pre, x, W1, W2, b2):
    n = cfg.n
    dinv = 1.0 / np.sqrt(pre["degp"])  # padded slots have deg=1
    xsp = np.zeros((cfg.npad, P), dtype=np.float32)
    xsp[pre["pad_id"](np.arange(n))] = np.asarray(x, np.float32)
    xsp *= dinv[:, None]
    xs = xsp.astype(ml_dtypes.bfloat16)

    iota = np.broadcast_to(np.arange(P, dtype=np.float32), (P, P)).astype(
        ml_dtypes.bfloat16
    )
    in_maps = []
    for c in range(cfg.n_cores):
        in_maps.append(
            {
                "xs": xs,
                "w1": np.asarray(W1, np.float32).astype(ml_dtypes.bfloat16),
                "w2": np.asarray(W2, np.float32).astype(ml_dtypes.bfloat16),
                "b2r": np.broadcast_to(np.asarray(b2, np.float32), (P, P)).copy(),
                "iota": np.ascontiguousarray(iota),
                "deg_own": pre["deg_all"][:, c * cfg.wpc : (c + 1) * cfg.wpc],
                "idx_all": pre["idx_all"][c],
                "rd_all": np.ascontiguousarray(pre["rd_all"][c]),
                "gcnt": pre["gcnt"][c][None, :],
            }
        )
    return in_maps


def _get_nc(cfg: Cfg, pre):
    key = (
        cfg,
        pre["icols"],
        pre["rcols"],
        pre["NI"].tobytes(),
        pre["G"].tobytes(),
    )
    if key not in _CACHE:
        _CACHE[key] = _build(cfg, pre)
    return _CACHE[key]


def _kernel_impl(cfg: Cfg, x, edge_index, W1, b1, W2, b2):
    from concourse.bass_utils import run_bass_kernel_spmd

    assert np.allclose(b1, 0.0), "kernel assumes b1 == 0 (spec fill: zeros)"

    pre = _preprocess(cfg, np.asarray(edge_index, dtype=np.int64))
    nc = _get_nc(cfg, pre)
    in_maps = _prep_inputs(cfg, pre, x, W1, W2, b2)

    res = run_bass_kernel_spmd(nc, in_maps, list(range(cfg.n_cores)))
    parts = [res.results[c]["out"][: cfg.npc] for c in range(cfg.n_cores)]
    return np.concatenate(parts, axis=0)


def kernel(x, edge_index, W1, b1, W2, b2):
    return _kernel_impl(CFG, x, edge_index, W1, b1, W2, b2)


# ----------------------------------------------------------------------------
# Steady-state timing support (no NTFF profiling under this axon client: we
# time repeated executions with device-resident inputs and subtract the
# dispatch floor measured with a null kernel).
# ----------------------------------------------------------------------------


def _make_runner(nc, n_cores):
    import jax
    from jax.sharding import Mesh, NamedSharding, PartitionSpec
    from jax.experimental.shard_map import shard_map

    from concourse import bass2jax, mybir

    bass2jax.install_neuronx_cc_hook()
    partition_name = nc.partition_id_tensor.name if nc.partition_id_tensor else None
    in_names, out_names, out_avals, zero_outs = [], [], [], []
    for alloc in nc.m.functions[0].allocations:
        if not isinstance(alloc, mybir.MemoryLocationSet):
            continue
        name = alloc.memorylocations[0].name
        if alloc.kind == "ExternalInput":
            if name != partition_name:
                in_names.append(name)
        elif alloc.kind == "ExternalOutput":
            shape = tuple(alloc.tensor_shape)
            dtype = mybir.dt.np(alloc.dtype)
            out_names.append(name)
            out_avals.append(jax.core.ShapedArray(shape, dtype))
            zero_outs.append(np.zeros(shape, dtype))
    n_params = len(in_names)
    all_in_names = list(in_names) + list(out_names)
    if partition_name is not None:
        all_in_names.append(partition_name)

    def _body(*args):
        operands = list(args)
        if partition_name is not None:
            operands.append(bass2jax.partition_id_tensor())
        outs = bass2jax._bass_exec_p.bind(
            *operands,
            out_avals=tuple(out_avals),
            in_names=tuple(all_in_names),
            out_names=tuple(out_names),
            lowering_input_output_aliases=(),
            sim_require_finite=True,
            sim_require_nnan=True,
            nc=nc,
        )
        return tuple(outs)

    devices = jax.devices()[:n_cores]
    mesh = Mesh(np.asarray(devices), ("core",))
    in_specs = (PartitionSpec("core"),) * (n_params + len(out_names))
    out_specs = (PartitionSpec("core"),) * len(out_names)
    fn = jax.jit(
        shard_map(
            _body, mesh=mesh, in_specs=in_specs, out_specs=out_specs, check_rep=False
        ),
        keep_unused=True,
    )
    sharding = NamedSharding(mesh, PartitionSpec("core"))

    staged = {}

    def run(in_maps, iters=1):
        import time as _t

        if "dev_in" not in staged:
            concat = [
                np.concatenate(
                    [np.asarray(in_maps[c][n]) for c in range(n_cores)], axis=0
                )
                for n in in_names
            ]
            concat += [
                np.zeros((n_cores * z.shape[0], *z.shape[1:]), z.dtype)
                for z in zero_outs
            ]
            staged["dev_in"] = [jax.device_put(a, sharding) for a in concat]
            outs = fn(*staged["dev_in"])
            jax.block_until_ready(outs)
        dev_in = staged["dev_in"]
        outs = fn(*dev_in)
        jax.block_until_ready(outs)
        times = []
        for _ in range(iters):
            t0 = _t.perf_counter()
            outs = fn(*dev_in)
            jax.block_until_ready(outs)
            times.append(_t.perf_counter() - t0)
        return outs, out_names, out_avals, times

    return run


def time_kernel(x, edge_index, W1, b1, W2, b2, iters=40, reps=9):
    """Interleave x1 and x{reps} executions so both see the same dispatch
    floor; the per-rep device time is the min of adjacent-pair deltas."""
    import time as _t

    cfg = CFG
    pre = _preprocess(cfg, np.asarray(edge_index, dtype=np.int64))
    in_maps = _prep_inputs(cfg, pre, x, W1, W2, b2)

    import jax

    def _stage(nc):
        run = _make_runner(nc, cfg.n_cores)
        # stage inputs + warm the compiled fn once
        return run

    run1 = _stage(_get_nc(cfg, pre))
    runR = _stage(_build(cfg, pre, reps=reps))

    # warm both, then interleave single timed executions
    _, _, _, _ = run1(in_maps, iters=2)
    _, _, _, _ = runR(in_maps, iters=2)
    t1s, tRs = [], []
    for _ in range(iters):
        _, _, _, a = run1(in_maps, iters=1)
        _, _, _, b = runR(in_maps, iters=1)
        t1s.append(a[0])
        tRs.append(b[0])
    deltas = [b - a for a, b in zip(t1s, tRs)]
    est = min(deltas) / (reps - 1)
    print(
        f"(x1 min {min(t1s)*1e3:.3f} med {sorted(t1s)[len(t1s)//2]*1e3:.3f} ms; "
        f"x{reps} min {min(tRs)*1e3:.3f} med {sorted(tRs)[len(tRs)//2]*1e3:.3f} ms; "
        f"delta min {min(deltas)*1e3:.3f} med {sorted(deltas)[len(deltas)//2]*1e3:.3f} ms)"
    )
    return est * 1e9
